# revision 1
# baseline (speedup 1.0000x reference)
"""Trainium2 Bass kernel for MaxTimesPlusErosionLiftingP4.

Reference math (per rotation i of 4, SAME padding, stride 1):
    out[b,i,h,w,f] = sum_c min_p (xpad[b,h+dy,w+dx,c] - kr_i[p,c,f]) / (tr_i[p,c,f]+eps)
where p = dy*7+dx (dy,dx in 0..6), kr_i/tr_i are the i-rotated (after 180-flip)
7x7xCxF kernels.

Device layout: the 4*32 = 128 (rotation, filter) combos live on the 128 SBUF
partitions; pixels live on the free axis.  Each of the 8 cores takes 32 of the
256 (b,h) rows.  The padded 3-channel slab (38 rows x 70 cols, zero halo) is
broadcast to all partitions, so every tap (p, c) is one [128, 32, 64] affine
v = (x_shift - k) * r  with per-partition scalars (k, r from the rotated
kernels), followed by a running elementwise min into a per-channel
accumulator.  Producers are split across ScalarE (activation Identity with
per-partition scale/bias) and VectorE (tensor_scalar sub+mul, 4x fp16 mode);
min-accumulation is split across VectorE (tensor_tensor min, 2x fp16 mode)
and GpSimd, with separate accumulator chains per (channel, engine) merged at
the end.  The channel sum is done in fp32 and DMA'd out contiguously; the
host reassembles [B,4,H,W,F].
"""

import os
from contextlib import ExitStack

import numpy as np

import concourse.bacc as bacc
import concourse.bass as bass
import concourse.mybir as mybir
import concourse.tile as tile
from concourse.bass_utils import run_bass_kernel_spmd

B, H, W, C, F = 4, 64, 64, 3, 32
KH = KW = 7
P = KH * KW  # 49 taps
NCORES = 8
ROWS = (B * H) // NCORES  # 32 output rows per core
HROWS = ROWS + KH - 1  # 38 slab rows (3 halo each side)
WPAD = W + KW - 1  # 70 slab cols (3 pad each side)
SLAB = C * HROWS * WPAD  # 7980 elements per channel-slab
NPIX = ROWS * W  # 2048 output pixels per core
NCOMB = 4 * F  # 128 (rotation, filter) combos -> partitions
NUNITS = P * C  # 147 tap-channel units
EPS = 1e-7

_DT = os.environ.get("EROSION_DT", "fp16")
# Producer split. GpSimd cannot do tensor-tensor min/max (walrus Pool codegen
# rejects it) and its tensor_scalar measured ~10x slower than modeled, so the
# tuned config produces everything on ScalarE (activation Identity, 16-bit
# packed ~2 elem/cyc/lane) while VectorE does every min-accumulate.
_PP = int(os.environ.get("EROSION_PP", 0))  # taps produced on GpSimd
_PD = int(os.environ.get("EROSION_PD", 0))  # taps produced on VectorE
_REPEAT = int(os.environ.get("EROSION_REPEAT", 1))  # benchmark: repeat compute
_NOMIN = int(os.environ.get("EROSION_NOMIN", 0))  # benchmark: skip min-accums
_BLOCKED = int(os.environ.get("EROSION_BLOCKED", 0))  # contiguous producer runs
_GROUP = int(os.environ.get("EROSION_GROUP", 0))  # tree-min group size (0 = chain)
_MERGEC = int(os.environ.get("EROSION_MERGEC", 1))  # one min op across all C channels
_VBUFS = int(os.environ.get("EROSION_VBUFS", 6))  # v-tile double-buffering depth
_MISALIGN = int(os.environ.get("EROSION_MISALIGN", 0))  # diagnostic: break 2x packing
_FLATMIN = int(os.environ.get("EROSION_FLATMIN", 1))  # flatten min operands to 2D
_MINOP = os.environ.get("EROSION_MINOP", "min")  # diagnostic: swap the accum ALU op

_cache = {}

last_results = None  # BassKernelResults of the most recent run (for test.py)


def _spread(total, count):
    """Deterministically spread `count` True flags over `total` slots."""
    return [((i + 1) * count) // total > (i * count) // total for i in range(total)]


def _build_module():
    dt = mybir.dt.float16 if _DT == "fp16" else mybir.dt.float32
    f32 = mybir.dt.float32

    nc = bacc.Bacc("TRN2", target_bir_lowering=False, debug=False)
    xs_d = nc.dram_tensor("xs", [SLAB + 4], dt, kind="ExternalInput")
    # k / r / b (= -k*r) per-partition scalar tables, concatenated so a single
    # DMA (one semaphore) covers all three.
    tabs_d = nc.dram_tensor("tabs", [NCOMB, 3 * NUNITS], f32, kind="ExternalInput")
    out_d = nc.dram_tensor("out", [NCOMB, NPIX], f32, kind="ExternalOutput")

    if _BLOCKED:
        prod_pool = [i < _PP for i in range(NUNITS)]
        prod_dve = [i >= NUNITS - _PD for i in range(NUNITS)]
    else:
        prod_pool = _spread(NUNITS, _PP)  # units produced on GpSimd
        prod_dve = _spread(NUNITS, _PD)  # units produced on VectorE (of the rest)

    with tile.TileContext(nc) as tc, ExitStack() as ctx:
        singles = ctx.enter_context(tc.tile_pool(name="singles", bufs=1))
        # Separate v pools per producer engine: keeps each slot's access
        # history on <=2 engines (producer + VectorE consumer) so waits stay
        # within the ISA limit after Bacc's event-semaphore splitting.
        vpools = {
            pr: ctx.enter_context(tc.tile_pool(name=f"v_{pr}", bufs=_VBUFS))
            for pr in ("act", "dve", "pool")
        }
        vgpool = ctx.enter_context(tc.tile_pool(name="vg", bufs=3))
        mpool = ctx.enter_context(tc.tile_pool(name="mpool", bufs=4))

        need_xo = _PD > 0
        xe = singles.tile([NCOMB, SLAB], dt, tag="xe")
        xo = singles.tile([NCOMB, SLAB], dt, tag="xo") if need_xo else None
        tabs = singles.tile([NCOMB, 3 * NUNITS], f32, tag="tabs")

        # Broadcast the slab to all 128 partitions (xo: same data shifted by
        # one element so odd-dx taps stay 4B-aligned for DVE packed modes).
        # Spread input DMAs over both HWDGE queues (SP + Activation).
        half = NCOMB // 2
        nc.sync.dma_start(
            out=xe[:half], in_=bass.AP(tensor=xs_d, offset=0, ap=[[0, half], [1, SLAB]])
        )
        nc.scalar.dma_start(
            out=xe[half:], in_=bass.AP(tensor=xs_d, offset=0, ap=[[0, half], [1, SLAB]])
        )
        nc.sync.dma_start(out=tabs[:], in_=tabs_d.ap())
        if xo is not None:
            nc.scalar.dma_start(
                out=xo[:], in_=bass.AP(tensor=xs_d, offset=1, ap=[[0, NCOMB], [1, SLAB]])
            )
        ktab = tabs[:, 0:NUNITS]
        rtab = tabs[:, NUNITS : 2 * NUNITS]
        btab = tabs[:, 2 * NUNITS : 3 * NUNITS]

        xe_r = xe[:].rearrange("p (c h w) -> p c h w", c=C, h=HROWS, w=WPAD)
        xo_r = (
            xo[:].rearrange("p (c h w) -> p c h w", c=C, h=HROWS, w=WPAD)
            if xo is not None
            else None
        )

        if _MERGEC:
            accm = singles.tile([NCOMB, C, ROWS, W], dt, tag="accm", name="accm")
            accs = {c: accm[:, c] for c in range(C)}
        else:
            accs = {}
            for c in range(C):
                accs[c] = singles.tile(
                    [NCOMB, ROWS, W], dt, tag=f"acc{c}", name=f"acc{c}"
                )

        def produce(j, c, dy, dx, target):
            """Emit one affine v = (x - k) * r into `target` on the unit's
            assigned producer engine."""
            sk = ktab[:, j : j + 1]
            sr = rtab[:, j : j + 1]
            sb = btab[:, j : j + 1]
            if prod_pool[j]:
                pr = "pool"
            elif prod_dve[j]:
                pr = "dve"
            else:
                pr = "act"
            if dx % 2 == 0 or pr != "dve":
                xsl = xe_r[:, c, dy : dy + ROWS, dx : dx + W]
            else:
                xsl = xo_r[:, c, dy : dy + ROWS, dx - 1 : dx - 1 + W]
            if pr == "dve":
                nc.vector.tensor_scalar(
                    target, xsl, sk, sr,
                    mybir.AluOpType.subtract, mybir.AluOpType.mult,
                )
            elif pr == "pool":
                nc.gpsimd.tensor_scalar(
                    target, xsl, sk, sr,
                    mybir.AluOpType.subtract, mybir.AluOpType.mult,
                )
            else:
                nc.scalar.activation(
                    out=target, in_=xsl,
                    func=mybir.ActivationFunctionType.Identity,
                    bias=sb, scale=sr,
                )
            return pr

        for _rep in range(_REPEAT):
            if _MERGEC:
                # One fused min per tap covering all 3 channels (FD = C*2048).
                for t in range(P):
                    dy, dx = divmod(t, KW)
                    if t == 0:
                        for c in range(C):
                            produce(t * C + c, c, dy, dx, accs[c])
                        continue
                    if _MISALIGN:
                        vcf = vpools["act"].tile(
                            [NCOMB, C * ROWS * W + 1], dt, tag="vc", name="vc"
                        )
                        vc = vcf[:, 1:].rearrange(
                            "p (c h w) -> p c h w", c=C, h=ROWS, w=W
                        )
                    else:
                        vc = vpools["act"].tile(
                            [NCOMB, C, ROWS, W], dt, tag="vc", name="vc"
                        )
                    for c in range(C):
                        produce(t * C + c, c, dy, dx, vc[:, c])
                    minop = getattr(mybir.AluOpType, _MINOP)
                    if _FLATMIN:
                        acc = accm[:].rearrange("p c h w -> p (c h w)")
                        vcf2 = vc.rearrange("p c h w -> p (c h w)")
                        nc.vector.tensor_tensor(acc, acc, vcf2, minop)
                    else:
                        acc = accm[:]
                        nc.vector.tensor_tensor(acc, acc, vc[:], minop)
            elif _GROUP == 0:
                started = set()
                for t in range(P):
                    dy, dx = divmod(t, KW)
                    for c in range(C):
                        j = t * C + c
                        first = c not in started
                        if first or _NOMIN:
                            started.add(c)
                            target = accs[c][:]
                        else:
                            pr = (
                                "pool" if prod_pool[j]
                                else ("dve" if prod_dve[j] else "act")
                            )
                            target = vpools[pr].tile(
                                [NCOMB, ROWS, W], dt, tag="v", name="v"
                            )[:]
                        produce(j, c, dy, dx, target)
                        if not first and not _NOMIN:
                            acc = accs[c][:]
                            nc.vector.tensor_tensor(
                                acc, acc, target, mybir.AluOpType.min
                            )
            else:
                G = _GROUP
                assert (P - 1) % G == 0, "group size must divide 48"
                ngroups = (P - 1) // G
                # tap 0 of each channel initializes the accumulator
                for c in range(C):
                    produce(0 * C + c, c, 0, 0, accs[c][:])
                for g in range(ngroups):
                    for c in range(C):
                        vg = vgpool.tile([NCOMB, G, ROWS, W], dt, tag="vg", name="vg")
                        for i in range(G):
                            t = 1 + g * G + i
                            dy, dx = divmod(t, KW)
                            produce(t * C + c, c, dy, dx, vg[:, i])
                        # pairwise-min tree inside the group
                        flat = vg[:].rearrange("p g h w -> p (g h w)")
                        n = G
                        level = 0
                        while n > 1:
                            s = 1 << level
                            view = flat.rearrange(
                                "p (m two s blk) -> p m two s blk",
                                two=2, s=s, blk=NPIX,
                            )
                            nc.vector.tensor_tensor(
                                view[:, :, 0, 0, :],
                                view[:, :, 0, 0, :],
                                view[:, :, 1, 0, :],
                                mybir.AluOpType.min,
                            )
                            n //= 2
                            level += 1
                        acc = accs[c][:]
                        nc.vector.tensor_tensor(
                            acc, acc, vg[:, 0], mybir.AluOpType.min
                        )

        # Channel sum, fully in fp32 (the min selections are exact picks of
        # fp16 v values; keeping the summation fp32 removes the largest
        # remaining rounding).
        s01 = mpool.tile([NCOMB, ROWS, W], f32, tag="s01", name="s01")[:]
        nc.vector.tensor_tensor(s01, accs[0][:], accs[1][:], mybir.AluOpType.add)
        out_t = singles.tile([NCOMB, ROWS, W], f32, tag="out", name="out_t")
        nc.vector.tensor_tensor(out_t[:], s01, accs[2][:], mybir.AluOpType.add)

        nc.sync.dma_start(out=out_d.ap(), in_=out_t[:])

    nc.compile()
    return nc


def _get_module():
    key = (
        _DT, _PD, _PP, _REPEAT, _NOMIN, _BLOCKED, _GROUP, _MERGEC, _VBUFS,
        _MISALIGN, _FLATMIN, _MINOP,
    )
    if key not in _cache:
        _cache[key] = _build_module()
    return _cache[key]


def _host_tables(kernel, timesKernel):
    k_ero = np.rot90(kernel, 2, axes=(0, 1))
    t_ero = np.rot90(timesKernel, 2, axes=(0, 1))
    K = np.zeros((P, C, NCOMB), np.float32)
    R = np.zeros((P, C, NCOMB), np.float32)
    for i in range(4):
        kr = np.rot90(k_ero, i, axes=(0, 1)).reshape(P, C, F)
        tr = np.rot90(t_ero, i, axes=(0, 1)).reshape(P, C, F)
        K[:, :, i * F : (i + 1) * F] = kr
        R[:, :, i * F : (i + 1) * F] = 1.0 / (tr + np.float32(EPS))
    # device layout [partition, unit] with unit j = tap*C + c
    Ktab = np.ascontiguousarray(K.reshape(NUNITS, NCOMB).T)
    Rtab = np.ascontiguousarray(R.reshape(NUNITS, NCOMB).T)
    Btab = np.ascontiguousarray(-(K * R).reshape(NUNITS, NCOMB).T)
    return Ktab, Rtab, Btab


def _host_slabs(x):
    np_dt = np.float16 if _DT == "fp16" else np.float32
    slabs = np.zeros((NCORES, SLAB + 4), np_dt)
    tmp = np.zeros((C, HROWS, WPAD), np.float32)
    for m in range(NCORES):
        b, half = divmod(m, 2)
        h0 = half * ROWS
        lo, hi = h0 - 3, h0 + ROWS + 3
        slo, shi = max(lo, 0), min(hi, H)
        tmp[:] = 0.0
        tmp[:, slo - lo : shi - lo, 3 : 3 + W] = np.transpose(
            x[b, slo:shi, :, :], (2, 0, 1)
        )
        slabs[m, :SLAB] = tmp.reshape(-1).astype(np_dt)
    return slabs


def kernel(x, kernel, timesKernel):
    global last_results
    x = np.asarray(x, np.float32)
    kernel = np.asarray(kernel, np.float32)
    timesKernel = np.asarray(timesKernel, np.float32)

    Ktab, Rtab, Btab = _host_tables(kernel, timesKernel)
    slabs = _host_slabs(x)

    nc = _get_module()
    tabs = np.ascontiguousarray(np.concatenate([Ktab, Rtab, Btab], axis=1))
    in_maps = [{"xs": slabs[m], "tabs": tabs} for m in range(NCORES)]
    res = run_bass_kernel_spmd(nc, in_maps, list(range(NCORES)))
    last_results = res

    full = np.zeros((B, 4, H, W, F), np.float32)
    for m in range(NCORES):
        b, half = divmod(m, 2)
        h0 = half * ROWS
        o = res.results[m]["out"].reshape(4, F, ROWS, W)
        full[b, :, h0 : h0 + ROWS, :, :] = np.transpose(o, (0, 2, 3, 1))
    return full

